# revision 22
# baseline (speedup 1.0000x reference)
"""Multi-head attention on 8 TRN2 NeuronCores.

Problem: queries [B,N,L,H,E], keys [B,N,S,H,E], values [B,N,S,H,D]
         out[b,n,l,h,:] = softmax(Q[b,n,l,h,:] @ K[b,n,:,h,:]^T / sqrt(E)) @ V[b,n,:,h,:]
with B,N,L,S,H,E,D = 4,7,512,512,8,64,64.

Sharding: head-parallel — core c computes all B*N=28 (b,n) slices for head h=c.

Device kernel per slice (L=S=512, E=D=64, P=128), all matmul operands fp16
(RNE-cast on host; scores/output accumulate in fp32 PSUM):
  1. scoresT [128s, 512l] chunks = K_sc^T (stationary) x Q^T (moving), two
     chunks per PSUM tile [128, 1024].
  2. attnT = exp(scores * 1/8) on ScalarE, one ACTIVATE per [128, 1024] pair
     (no max-subtraction: |scores|/8 <= ~6, exp fits fp16/fp32 comfortably).
  3. po [128, 512] += VA_sc (stationary) x attnT_sc (moving) where
     VA = [V | ones | 0-pad] so row 64 of po is the softmax denominator.
  4. rrow = 1/po[64] (VectorE), broadcast across partitions (GpSimd),
     osb = po[0:64] * rbc (VectorE), DMA out as [64, 512] (d-major; host
     transposes back to [l, d] while unsharding).

Software-pipelined one slice deep so the PE never waits on ScalarE's exp.
"""

import numpy as np

B, N, L, S, H, E, D = 4, 7, 512, 512, 8, 64, 64
NS = B * N          # 28 (b,n) slices per core
NP = NS // 2        # 14 slice-pairs
P = 128
SC = S // P         # 4 s-chunks
SCALE = 1.0 / float(np.sqrt(E))

# input pack layout (fp16), per slice-pair: [128, 2048] =
#   [0:512)     qtT pair  (rows 0-63 = slice a's [E, L], rows 64-127 = slice b)
#   [512:1024)  ktT pair  (same row split, cols = S)
#   [1024:1536) VA slice a: 4 s-chunks x 128 cols = [V | ones | zeros]
#   [1536:2048) VA slice b
QOFF, KOFF, VOFF = 0, 512, 1024

_CACHE = {}


def _build_program():
    import concourse.mybir as mybir
    import concourse.tile as tile
    from concourse import bacc
    import concourse.bass as bass

    f32 = mybir.dt.float32
    f16 = mybir.dt.float16
    Exp = mybir.ActivationFunctionType.Exp

    nc = bacc.Bacc("TRN2", target_bir_lowering=False, debug=False)
    inp = nc.dram_tensor("inp", [NP, P, 2048], f16, kind="ExternalInput").ap()
    o = nc.dram_tensor("o", [NS, D, L], f32, kind="ExternalOutput").ap()

    with tile.TileContext(nc) as tc:
        with (
            tc.tile_pool(name="inpool", bufs=4) as in_pool,
            tc.tile_pool(name="attn", bufs=6) as at_pool,
            tc.tile_pool(name="rrow", bufs=3) as r_pool,
            tc.tile_pool(name="rbc", bufs=3) as rbc_pool,
            tc.tile_pool(name="osb", bufs=3) as osb_pool,
            tc.tile_pool(name="ps", bufs=2, space=bass.MemorySpace.PSUM) as ps_pool,
            tc.tile_pool(name="po", bufs=4, space=bass.MemorySpace.PSUM) as po_pool,
        ):
            def emit_pv_epilogue(state):
                in_t, j, ats, i = state
                po = po_pool.tile([P, L], f32)
                for sc in range(SC):
                    nc.tensor.matmul(
                        po[:],
                        lhsT=in_t[:, VOFF + j * 512 + sc * P: VOFF + j * 512 + (sc + 1) * P],
                        rhs=ats[sc // 2][:, (sc % 2) * L:(sc % 2 + 1) * L],
                        start=(sc == 0),
                        stop=(sc == SC - 1),
                    )
                # VA = [ones | 0*63 | V]: po[0] = denom (partition 0 — the
                # custom-DVE recip mishandles nonzero partition offsets),
                # po[64:128] = numerator^T (32-aligned partition start).
                # po pool is 4 deep, so the ~2.6us recip->broadcast->mul trail
                # on po never gates the PE's next-slice PSUM allocation.
                rrow = r_pool.tile([1, L], f32)
                nc.vector.reciprocal_approx_fast(rrow[:], po[0:1, :])
                rbc = rbc_pool.tile([D, L], f32)
                nc.gpsimd.partition_broadcast(rbc[:], rrow[:])
                osb = osb_pool.tile([D, L], f32)
                nc.vector.tensor_mul(osb[:], po[D:2 * D, :], rbc[:])
                nc.sync.dma_start(o[i], osb[:])

            # PV/epilogue run two slices behind the QK/exp front so the PE
            # never waits on ScalarE's exp and the opening QK burst is long
            # enough (>3.4us continuous) to open the HAM clock gate.
            pending = []
            for pair in range(NP):
                in_t = in_pool.tile([P, 2048], f16)
                nc.sync.dma_start(in_t[:], inp[pair])
                if pair == 0:
                    # HAM warm-up insurance: back-to-back dummy matmuls give
                    # the PE one full 3.4us continuous-busy window right away
                    # so the clock gate opens (1.2 -> 2.4 GHz) before the
                    # steady-state pipeline (with its short stalls) begins.
                    wps = ps_pool.tile([P, L], f32, tag="ps")
                    for _ in range(9):
                        nc.tensor.matmul(
                            wps[:],
                            lhsT=in_t[:, VOFF:VOFF + P],
                            rhs=in_t[:, 0:L],
                            start=True,
                            stop=True,
                        )
                for j in range(2):
                    i = 2 * pair + j
                    rq = in_t[j * E:(j + 1) * E, QOFF:QOFF + L]
                    ats = []
                    for half in range(2):
                        ps = ps_pool.tile([P, 2 * L], f32)
                        for k in range(2):
                            sc = 2 * half + k
                            nc.tensor.matmul(
                                ps[:, k * L:(k + 1) * L],
                                lhsT=in_t[j * E:(j + 1) * E, KOFF + sc * P:KOFF + (sc + 1) * P],
                                rhs=rq,
                                start=True,
                                stop=True,
                            )
                        at = at_pool.tile([P, 2 * L], f16)
                        nc.scalar.activation(at[:], ps[:], Exp, scale=SCALE)
                        ats.append(at)
                    pending.append((in_t, j, ats, i))
                    if len(pending) > 2:
                        emit_pv_epilogue(pending.pop(0))
            for state in pending:
                emit_pv_epilogue(state)
    nc.compile()
    return nc


def _prep_inputs(queries, keys, values):
    """Pack per-core fp16 inputs. Core c gets head h=c."""
    q = np.asarray(queries, dtype=np.float32)
    k = np.asarray(keys, dtype=np.float32)
    v = np.asarray(values, dtype=np.float32)

    # [H, NP, 128, 512] — Q^T/K^T per slice, slice-pairs stacked on partitions
    qt = np.ascontiguousarray(q.transpose(3, 0, 1, 4, 2)).reshape(H, NP, P, L)
    kt = np.ascontiguousarray(k.transpose(3, 0, 1, 4, 2)).reshape(H, NP, P, S)

    # VA: [H, NS, SC, 128 s, 128 cols] = [ones | zeros | V] -> [H, NP, 128, 1024]
    va = np.zeros((H, NS, SC, P, P), dtype=np.float32)
    va[..., D:2 * D] = v.transpose(3, 0, 1, 2, 4).reshape(H, NS, SC, P, D)
    va[..., 0] = 1.0
    va = va.transpose(0, 1, 3, 2, 4).reshape(H, NP, 2, P, SC * P)
    va = np.ascontiguousarray(va.transpose(0, 1, 3, 2, 4)).reshape(H, NP, P, 2 * SC * P)

    inp = np.concatenate([qt, kt, va], axis=-1).astype(np.float16)
    return [{"inp": inp[c]} for c in range(H)]


def _run(in_maps, trace=False, tmpdir=None):
    from concourse.bass_utils import run_bass_kernel_spmd

    if "nc" not in _CACHE:
        _CACHE["nc"] = _build_program()
    kwargs = {}
    if tmpdir is not None:
        kwargs["tmpdir"] = tmpdir
    return run_bass_kernel_spmd(
        _CACHE["nc"], in_maps, core_ids=list(range(H)), trace=trace, **kwargs
    )


def kernel(queries, keys, values, _trace=False, _results_out=None, _tmpdir=None):
    in_maps = _prep_inputs(queries, keys, values)
    res = _run(in_maps, trace=_trace, tmpdir=_tmpdir)
    if _results_out is not None:
        _results_out.append(res)
    # res.results[c]["o"]: [NS, D, L] for head c  ->  [B, N, L, H, D]
    out = np.stack([res.results[c]["o"] for c in range(H)], axis=0)
    out = out.reshape(H, B, N, D, L).transpose(1, 2, 4, 0, 3)
    return np.ascontiguousarray(out)
